# revision 11
# baseline (speedup 1.0000x reference)
"""Trainium2 Bass kernel for nn_Encoder_29712583754153.

Model: multi-hot embedding-sum (duplicates collapse, code 0 = padding) ->
tanh -> length-masked GRU over S steps -> tanh latent head. Batch is
sorted by length descending (output stays sorted).

Sharding: data-parallel over the batch; core c owns sorted patients
[8c, 8c+8). No collectives. Embedding rows are gathered with the SWDGE
dma_gather (transpose mode, bf16) so the embedding dim lands on SBUF
partitions; the 32 codes of a visit are summed with a DVE add-tree.
Gate pre-activations gi for all visits come from one matmul whose
contraction is augmented with a ones-row (bias) and a padded-visit
indicator row that adds +30 to the z gate (sigmoid -> 1) wherever
t >= length, which freezes h exactly like the reference mask.
The GRU recurrence keeps Wh stationary on the PE (12 M-chunks x 4
K-chunks of [128,128] bf16 tiles) so gates land transposed
([3H-chunks on partitions, patients on free]) where the vector/scalar
engine work is cheap.
"""

import sys

for _p in ("/opt/trn_rl_repo",):
    if _p not in sys.path:
        sys.path.insert(0, _p)

import numpy as np
import ml_dtypes

BF16 = ml_dtypes.bfloat16

# Problem dims (hardcoded per the contract).
B, S, K = 64, 80, 32
V, E, H, L = 10000, 256, 512, 128
H3 = 3 * H
NCORES = 8
BL = B // NCORES  # patients per core
ZROW = V          # index of the all-zeros embedding row
VPAD = V + 16     # padded embedding-table rows

EC = E // 128     # 2   e-dim chunks
HC = H // 128     # 4   h-dim chunks
GC = H3 // 128    # 12  gate-dim chunks


# --------------------------------------------------------------------------
# Host-side input prep
# --------------------------------------------------------------------------

def _prep_shared(embedding_weight, Wi, Wh, bi, bh, W_lat, b_lat):
    """Weights shared by all cores (numpy, device-ready layouts)."""
    emb = np.zeros((VPAD, E), dtype=BF16)
    emb[:V] = embedding_weight.astype(BF16)

    wiT = np.ascontiguousarray(Wi.T).astype(BF16)      # [E, 3H]
    whT = np.ascontiguousarray(Wh.T).astype(BF16)      # [H, 3H]

    bias_row = (bi + bh).astype(np.float64)
    bias_row[2 * H:] = bi[2 * H:]                      # bh_n stays inside r*(.)
    zmask_row = np.zeros(H3)
    zmask_row[H:2 * H] = 30.0                          # +30 on z rows when padded
    extw = np.stack([bias_row, zmask_row]).astype(BF16)  # [2, 3H]

    bhn = np.ascontiguousarray(bh[2 * H:].reshape(HC, 128).T)      # [128, HC]
    bhn = np.repeat(bhn[:, :, None], BL, axis=2).astype(np.float32)  # [128,HC,BL]

    pat = np.zeros((128, 252), dtype=BF16)             # sliding K-sum pattern
    pat[np.arange(128), 124 + np.arange(128) // 32] = 1.0

    wlatT = np.ascontiguousarray(W_lat.T).astype(BF16)  # [H, L]
    blat = b_lat.reshape(L, 1).astype(np.float32)       # [128, 1]
    return dict(emb=emb, wiT=wiT, whT=whT, extw=extw, bhn=bhn,
                wlatT=wlatT, blat=blat, pat=pat)


def _prep_core(seq_s, len_s, c, s_steps=S, bl=BL):
    """Per-core gather indices + padded-visit row. seq_s/len_s are sorted.

    Gather plan: one indirect DMA per 128 rows (= 4 visits x 32 codes);
    call j's partition p fetches code slot p%32 of visit 4j + p//32
    (visits ordered s-major: v = s*bl + patient).
    """
    sl = seq_s[c * bl:(c + 1) * bl, :s_steps]          # [bl, s, K]
    ll = len_s[c * bl:(c + 1) * bl]                    # [bl]
    x = np.asarray(sl, dtype=np.int64)

    dup = np.zeros(x.shape, dtype=bool)
    for k in range(1, K):
        dup[:, :, k] = (x[:, :, :k] == x[:, :, k:k + 1]).any(-1)
    in_len = np.arange(s_steps)[None, :, None] < np.asarray(ll)[:, None, None]
    valid = (~dup) & (x != 0) & in_len
    idx = np.where(valid, x, ZROW)                     # [bl, s, K]

    vis = bl * s_steps
    zidx = idx.transpose(1, 0, 2).reshape(vis, K)      # s-major visits
    ncall = vis // 4
    vv = 4 * np.arange(ncall)[None, :] + (np.arange(128) // 32)[:, None]
    kk = np.broadcast_to((np.arange(128) % 32)[:, None], vv.shape)
    idx32 = zidx[vv, kk].astype(np.int32)              # [128, ncall]

    padv = (np.arange(s_steps)[:, None] >= np.asarray(ll)[None, :]).reshape(-1)
    extra2 = np.stack([np.ones(vis), padv.astype(np.float64)]).astype(BF16)
    return dict(idx32=idx32, extra2=extra2)


# --------------------------------------------------------------------------
# Bass kernel builder
# --------------------------------------------------------------------------

def build_nc(s_steps=S, bl=BL, nchunk=4, debug=False):
    """Build the per-core Bass program (SPMD: all cores run this NEFF)."""
    import concourse.bass as bass
    import concourse.mybir as mybir
    import concourse.tile as tile
    from concourse import bacc

    fp32 = mybir.dt.float32
    bf16 = mybir.dt.bfloat16
    AF = mybir.ActivationFunctionType

    vis = bl * s_steps           # visits per core
    VG = 128 if vis % 128 == 0 else vis   # visits per pipeline group
    ngrp = vis // VG
    NCg = VG // 4                # indirect-DMA calls per group
    ncall = vis // 4
    steps_per_grp = VG // bl

    nc = bacc.Bacc("TRN2", target_bir_lowering=False, debug=debug,
                   enable_asserts=False, num_devices=1)

    # ---- DRAM I/O ----
    emb_d = nc.dram_tensor("emb", [VPAD, E], bf16, kind="ExternalInput")
    idx_d = nc.dram_tensor("idx32", [128, ncall], mybir.dt.int32,
                           kind="ExternalInput")
    pat_d = nc.dram_tensor("pat", [128, 252], bf16, kind="ExternalInput")
    wiT_d = nc.dram_tensor("wiT", [E, H3], bf16, kind="ExternalInput")
    extw_d = nc.dram_tensor("extw", [2, H3], bf16, kind="ExternalInput")
    extra2_d = nc.dram_tensor("extra2", [2, vis], bf16, kind="ExternalInput")
    whT_d = nc.dram_tensor("whT", [H, H3], bf16, kind="ExternalInput")
    bhn_d = nc.dram_tensor("bhn", [128, HC, bl], fp32, kind="ExternalInput")
    wlatT_d = nc.dram_tensor("wlatT", [H, L], bf16, kind="ExternalInput")
    blat_d = nc.dram_tensor("blat", [L, 1], fp32, kind="ExternalInput")
    out_d = nc.dram_tensor("outT", [L, bl], fp32, kind="ExternalOutput")

    with tile.TileContext(nc) as tc:
        with (
            tc.tile_pool(name="const", bufs=1) as cpool,
            tc.tile_pool(name="gbuf", bufs=2) as gpool,
            tc.tile_pool(name="state", bufs=2) as spool,
            tc.tile_pool(name="psum", bufs=2, space="PSUM") as ppool,
            tc.tile_pool(name="psum_x", bufs=1, space="PSUM") as xpool,
            tc.tile_pool(name="psum_gi", bufs=2, space="PSUM") as gipool,
        ):
            # ---- resident tensors ----
            idx_sb = cpool.tile([128, ncall], mybir.dt.int32)
            nc.sync.dma_start(idx_sb[:], idx_d[:])
            pat_sb = cpool.tile([128, 252], bf16)
            nc.sync.dma_start(pat_sb[:], pat_d[:])

            wiT_sb = cpool.tile([128, EC, H3], bf16)
            nc.sync.dma_start(
                wiT_sb[:], wiT_d.rearrange("(c p) m -> p c m", p=128))
            extw_sb = cpool.tile([2, H3], bf16)
            nc.sync.dma_start(extw_sb[:], extw_d[:])
            extra2_sb = cpool.tile([2, vis], bf16)
            nc.sync.dma_start(extra2_sb[:], extra2_d[:])
            whT_sb = cpool.tile([128, HC, H3], bf16)
            nc.sync.dma_start(
                whT_sb[:], whT_d.rearrange("(c p) m -> p c m", p=128))
            bhn_sb = cpool.tile([128, HC, bl], fp32)
            nc.sync.dma_start(bhn_sb[:], bhn_d[:])
            wlatT_sb = cpool.tile([128, HC, L], bf16)
            nc.sync.dma_start(
                wlatT_sb[:], wlatT_d.rearrange("(c p) m -> p c m", p=128))
            blat_sb = cpool.tile([128, 1], fp32)
            nc.sync.dma_start(blat_sb[:], blat_d[:])

            xeT_g = [cpool.tile([128, EC, VG], bf16, tag=f"xeT{g}",
                                name=f"xeT{g}") for g in range(ngrp)]
            giT_g = [cpool.tile([128, GC, VG], bf16, tag=f"giT{g}",
                                name=f"giT{g}") for g in range(ngrp)]

            def emit_embed(g):
                gb = gpool.tile([128, NCg, E], bf16, tag="g")
                for c in range(NCg):
                    j = g * NCg + c
                    nc.gpsimd.indirect_dma_start(
                        out=gb[:, c, :], out_offset=None, in_=emb_d[:],
                        in_offset=bass.IndirectOffsetOnAxis(
                            ap=idx_sb[:, j:j + 1], axis=0))
                ps = xpool.tile([128, E], fp32, tag="psx")
                for c in range(NCg):
                    nc.tensor.matmul(
                        ps[:], pat_sb[:, 124 - 4 * c:252 - 4 * c], gb[:, c, :],
                        start=(c == 0), stop=(c == NCg - 1))
                xe = gpool.tile([128, E], bf16, tag="xe")
                nc.scalar.activation(xe[:VG, :], ps[:VG, :], AF.Tanh)
                for j in range(EC):
                    nc.sync.dma_start_transpose(
                        xeT_g[g][:, j, :], xe[:VG, j * 128:(j + 1) * 128])

            def emit_gi(g):
                vs = slice(g * VG, (g + 1) * VG)
                for m in range(GC):
                    ms = slice(m * 128, (m + 1) * 128)
                    ps = gipool.tile([128, VG], fp32, tag="psgi")
                    for kc in range(EC):
                        nc.tensor.matmul(
                            ps[:], wiT_sb[:, kc, ms], xeT_g[g][:, kc, :],
                            start=(kc == 0), stop=False)
                    nc.tensor.matmul(
                        ps[:], extw_sb[:, ms], extra2_sb[:, vs],
                        start=False, stop=True)
                    nc.vector.tensor_copy(giT_g[g][:, m, :], ps[:])

            # ---- GRU over s_steps ----
            h32 = spool.tile([128, HC, bl], fp32, tag="h32")
            hbf = spool.tile([128, HC, bl], bf16, tag="hbf")
            nc.vector.memset(h32[:], 0.0)
            nc.vector.memset(hbf[:], 0.0)

            def emit_gru_step(t):
                nonlocal h32, hbf
                ps_rz = ppool.tile([128, 8, bl], fp32, tag="psrz")
                ps_n = ppool.tile([128, HC, bl], fp32, tag="psn")
                for m in range(8):
                    for kc in range(HC):
                        nc.tensor.matmul(
                            ps_rz[:, m, :],
                            whT_sb[:, kc, m * 128:(m + 1) * 128],
                            hbf[:, kc, :],
                            start=(kc == 0), stop=(kc == HC - 1))
                for m in range(HC):
                    for kc in range(HC):
                        nc.tensor.matmul(
                            ps_n[:, m, :],
                            whT_sb[:, kc, (8 + m) * 128:(9 + m) * 128],
                            hbf[:, kc, :],
                            start=(kc == 0), stop=(kc == HC - 1))

                gt, lo = (t * bl) // VG, (t * bl) % VG
                gs = giT_g[gt][:, :, lo:lo + bl]         # [128, GC, bl]
                rzp = spool.tile([128, 8, bl], fp32, tag="rzp")
                nc.vector.tensor_add(rzp[:], gs[:, 0:8, :], ps_rz[:])
                rz = spool.tile([128, 8, bl], fp32, tag="rz")
                nc.scalar.activation(rz[:], rzp[:], AF.Sigmoid)

                nb = spool.tile([128, HC, bl], fp32, tag="nb")
                nc.vector.tensor_add(nb[:], ps_n[:], bhn_sb[:])
                nm = spool.tile([128, HC, bl], fp32, tag="nm")
                nc.vector.tensor_mul(nm[:], rz[:, 0:HC, :], nb[:])
                np_ = spool.tile([128, HC, bl], fp32, tag="npre")
                nc.vector.tensor_add(np_[:], nm[:], gs[:, 8:12, :])
                n_ = spool.tile([128, HC, bl], fp32, tag="n")
                nc.scalar.activation(n_[:], np_[:], AF.Tanh)

                dd = spool.tile([128, HC, bl], fp32, tag="dd")
                nc.vector.tensor_sub(dd[:], h32[:], n_[:])
                zd = spool.tile([128, HC, bl], fp32, tag="zd")
                nc.vector.tensor_mul(zd[:], rz[:, HC:8, :], dd[:])
                h32n = spool.tile([128, HC, bl], fp32, tag="h32")
                nc.vector.tensor_add(h32n[:], n_[:], zd[:])
                hbfn = spool.tile([128, HC, bl], bf16, tag="hbf")
                nc.vector.tensor_copy(hbfn[:], h32n[:])
                h32, hbf = h32n, hbfn

            # ---- pipelined schedule: embed(g) -> gi(g) -> GRU steps of g;
            #      group g+1's gathers run on GPSIMD under group g's GRU ----
            for g in range(ngrp):
                emit_embed(g)
                emit_gi(g)
                for t in range(g * steps_per_grp, (g + 1) * steps_per_grp):
                    emit_gru_step(t)

            # ---- head ----
            ps_o = ppool.tile([128, bl], fp32, tag="pso", bufs=1)
            for kc in range(HC):
                nc.tensor.matmul(ps_o[:], wlatT_sb[:, kc, :], hbf[:, kc, :],
                                 start=(kc == 0), stop=(kc == HC - 1))
            outT_sb = cpool.tile([128, bl], fp32)
            nc.scalar.activation(outT_sb[:], ps_o[:], AF.Tanh, bias=blat_sb[:])
            nc.sync.dma_start(out_d[:], outT_sb[:])

    nc.compile()
    return nc


# --------------------------------------------------------------------------
# Host entry
# --------------------------------------------------------------------------

def make_in_maps(inputs, s_steps=S, bl=BL):
    """Full inputs -> (per-core in_maps, sort order)."""
    seq = np.asarray(inputs["input_sequence"], dtype=np.int64)
    ln = np.asarray(inputs["length"], dtype=np.int64)
    order = np.argsort(-ln, kind="stable")
    seq_s, len_s = seq[order], ln[order]

    shared = _prep_shared(
        np.asarray(inputs["embedding_weight"], dtype=np.float32),
        np.asarray(inputs["Wi"], dtype=np.float32),
        np.asarray(inputs["Wh"], dtype=np.float32),
        np.asarray(inputs["bi"], dtype=np.float32),
        np.asarray(inputs["bh"], dtype=np.float32),
        np.asarray(inputs["W_lat"], dtype=np.float32),
        np.asarray(inputs["b_lat"], dtype=np.float32),
    )
    ncores = seq_s.shape[0] // bl
    in_maps = []
    for c in range(ncores):
        per = _prep_core(seq_s, len_s, c, s_steps=s_steps, bl=bl)
        in_maps.append({
            "emb": shared["emb"], "idx32": per["idx32"],
            "pat": shared["pat"],
            "wiT": shared["wiT"], "extw": shared["extw"],
            "extra2": per["extra2"], "whT": shared["whT"],
            "bhn": shared["bhn"], "wlatT": shared["wlatT"],
            "blat": shared["blat"],
        })
    return in_maps, order


_NC_CACHE = {}


def _get_nc():
    key = (S, BL)
    if key not in _NC_CACHE:
        _NC_CACHE[key] = build_nc()
    return _NC_CACHE[key]


def kernel(**inputs) -> np.ndarray:
    from concourse.bass_utils import run_bass_kernel_spmd

    nc = _get_nc()
    in_maps, _order = make_in_maps(inputs)
    res = run_bass_kernel_spmd(nc, in_maps, core_ids=list(range(NCORES)))
    outs = [np.ascontiguousarray(r["outT"].T) for r in res.results]
    return np.concatenate(outs, axis=0).astype(np.float32)
